# revision 31
# baseline (speedup 1.0000x reference)
"""GQA kernel for trn2, 8 NeuronCores.

Problem: nn_GroupedQueryAttention (b=4, s=2048, 16 q-heads / 4 kv-heads, d=64).
The reference's score einsum 'bghsd,bhad->bhsa' SUMS over the group axis g, and
RoPE is linear in x, so the module collapses to 4-head MHA with Wq pre-summed
over groups.

Sharding: 8 cores = (batch b in 0..3) x (head-group hg in 0..1, 2 heads each).
Each core computes its two heads' attention output and a partial product with
its 128-row slice of Wo; the host sums the two partials per batch.

Perf design (vs the f32 baseline at ~381us):
 - All matmuls at 1 cycle/row: projections, attn@V and Wo run in bf16;
   scores run f32->f32r bitcast (wide moving dim). f32 costs 4 cycles/row.
 - RoPE swap: instead of a second full projection with column-swapped weights,
   the projected chunk is copied to SBUF (bf16, Pool engine) and multiplied by
   a 128x128 permutation matrix (1 matmul of 512 rows vs 8).
 - Causal mask: diagonal-crossing tiles compute only the causally-needed
   query range (narrowed N); the per-tile 128x128 triangle is zeroed post-exp
   on DVE with a bf16 upper-tri mask. No mask-inject matmuls.
 - exp batching: score tiles for two key-tiles land side by side in a 2-bank
   PSUM tile so one ACT instruction handles 1024 columns (ACT per-instruction
   bubble is ~370ns).
 - Wo partials go PSUM -> DRAM directly (no staging copies); Wo matmuls are
   interleaved into the attention instruction stream, and attn@V runs one
   iteration behind scores (software pipelining) so PE never waits for exp.
 - softmax denominators come from a ones-column appended to V; normalization
   reciprocal is broadcast across partitions via a DRAM round-trip.
"""

from collections import deque

import numpy as np

B, S, IN_DIM = 4, 2048, 1024
Q_HEADS, KV_HEADS, HEAD_DIM = 16, 4, 64
GROUPS = Q_HEADS // KV_HEADS
HALF = HEAD_DIM // 2  # 32
N_CORES = 8
SC = 512  # s-chunk width (psum bank)
AT = 128  # a-tile width

_cached = {}


def _install_wait_splitter():
    """This walrus build accepts only ONE semaphore wait per instruction.
    Tile emits several; hoist all-but-one into standalone EventSemaphores."""
    import concourse.mybir as mybir
    import concourse.tile as tile
    from concourse._compat import not_none as nn

    if getattr(tile.TileContext, "_wait_split_installed", False):
        return

    orig_add = tile.TileContext._add_instruction

    def patched_add(self, inst):
        si = getattr(inst, "sync_info", None)
        if si is not None and si.on_wait and len(si.on_wait) > 1:
            waits = list(si.on_wait)
            for w in waits[:-1]:
                nm = self.nc.get_next_instruction_name()
                ev = mybir.InstEventSemaphore(
                    name=nm, engine=inst.engine, ins=[], outs=[],
                    sync_info=mybir.SyncInfo(on_wait=[w], on_update=[]))
                orig_add(self, ev)
            inst.sync_info = mybir.SyncInfo(
                on_wait=[waits[-1]], on_update=list(si.on_update or []))
        orig_add(self, inst)

    def patched_drain(self, tick_clock, wait_clock):
        # reimplementation of the original: same drain -> barrier -> sem-clear
        # -> barrier sequence, but the drain's (many) waits are split into
        # standalone EventSemaphores emitted BEFORE the sem clear.
        from concourse.vector_clock import ScopedClock

        nc = self.nc
        drain_wrap = nc.sync.drain()
        drain_inst = drain_wrap.ins  # BassInstruction wrapper -> mybir inst
        wait_clock.add_sem_waits(
            drain_inst, ScopedClock({None: tick_clock.global_clock}))
        bb = nn(nc.cur_bb).bb
        si = getattr(drain_inst, "sync_info", None)
        if si is not None and si.on_wait and len(si.on_wait) > 1:
            waits = list(si.on_wait)
            drain_inst.sync_info = mybir.SyncInfo(
                on_wait=[waits[0]], on_update=list(si.on_update or []))
            for w in waits[1:]:
                nm = nc.get_next_instruction_name()
                ev = mybir.InstEventSemaphore(
                    name=nm, engine=drain_inst.engine, ins=[], outs=[],
                    sync_info=mybir.SyncInfo(on_wait=[w], on_update=[]))
                nc.register_instruction(ev, overwrite=True)
                bb.add_instruction(ev)

        nc.all_engine_barrier()
        assert self.sems is not None
        popped = nc._tile_sem_poison_stack.pop()
        assert popped is self._sem_poison
        nc.clear_and_free_semaphores(list(self.sems.allocated().values()))
        nc.all_engine_barrier()

    tile.TileContext._add_instruction = patched_add
    tile.TileContext._drain_and_barrier = patched_drain
    tile.TileContext._wait_split_installed = True


def _build_nc():
    import concourse.bass as bass
    import concourse.mybir as mybir
    import concourse.tile as tile

    _install_wait_splitter()

    f32 = mybir.dt.float32
    f32r = mybir.dt.float32r
    bf16 = mybir.dt.bfloat16
    EXP = mybir.ActivationFunctionType.Exp
    SCALE = float(1.0 / np.sqrt(HEAD_DIM))

    nc = bass.Bass()

    qT = nc.declare_dram_parameter("qT", [4 * 128, 8 * 512], bf16,
                               isOutput=False)  # [c, p, t, s] pre-tiled
    wq = nc.declare_dram_parameter("wq", [IN_DIM, 128], bf16, isOutput=False)
    wk = nc.declare_dram_parameter("wk", [IN_DIM, 128], bf16, isOutput=False)
    wv = nc.declare_dram_parameter("wv", [IN_DIM, 128], bf16, isOutput=False)
    wo = nc.declare_dram_parameter("wo", [128, IN_DIM], bf16, isOutput=False)
    perm = nc.declare_dram_parameter("perm", [128, 128], bf16, isOutput=False)
    tri = nc.declare_dram_parameter("tri", [128, 128], bf16, isOutput=False)
    cc = nc.declare_dram_parameter("cc", [128, S], f32, isOutput=False)
    ss = nc.declare_dram_parameter("ss", [128, S], f32, isOutput=False)
    out = nc.declare_dram_parameter("out", [S, IN_DIM], bf16, isOutput=True)

    NSC = S // SC       # 4 s-chunks
    NAT = S // AT       # 16 a-tiles
    NIT = IN_DIM // 128  # 8 i-tiles

    def r32(ap):
        return ap.bitcast(f32r)

    with tile.TileContext(nc) as tc:
        with (
            tc.tile_pool(name="big", bufs=1) as big,
            tc.tile_pool(name="psum", bufs=1, space="PSUM") as psum,
            tc.tile_pool(name="sbx", bufs=2) as sbxp,
            tc.tile_pool(name="tmp", bufs=3) as tmp,
            tc.tile_pool(name="atp", bufs=1) as atp,
            tc.tile_pool(name="avp", bufs=2) as avp,
            tc.tile_pool(name="recp", bufs=2) as recp,
            tc.tile_pool(name="dram", bufs=2, space="DRAM") as dram,
        ):
            # ---- resident SBUF tensors ----
            qT_sb = big.tile([128, NSC, NIT, SC], bf16)
            wq_sb = big.tile([128, NIT, 128], bf16)
            wk_sb = big.tile([128, NIT, 128], bf16)
            wv_sb = big.tile([128, NIT, 128], bf16)
            wo_sb = big.tile([128, IN_DIM], bf16)
            perm_sb = big.tile([128, 128], bf16)
            tri_sb = big.tile([128, 128], bf16)
            cc_sb = big.tile([128, S], f32)
            ss_sb = big.tile([128, S], f32)
            qh_sb = big.tile([128, S], bf16)  # roped q, [2 heads x (32r|32i)], s
            kh_sb = big.tile([128, S], bf16)
            v_sb = big.tile([128, 2, NAT, HEAD_DIM + 1], bf16)  # [a, h, t, d+1]
            on_sb = big.tile([128, S], bf16)  # normalized outT, 2 heads stacked
            ones_sb = big.tile([1, HEAD_DIM], f32r)
            ones_f32 = big.tile([1, HEAD_DIM], f32)

            def dma_qt(c, split=1):
                w = NIT // split
                for u in range(split):
                    nc.sync.dma_start(
                        out=qT_sb[:, c, u * w:(u + 1) * w, :],
                        in_=qT[c * 128:(c + 1) * 128,
                               u * w * SC:(u + 1) * w * SC].rearrange(
                            "p (t s) -> p t s", s=SC))

            def dma_w(w_sb, w):
                nc.sync.dma_start(
                    out=w_sb, in_=w.rearrange("(t p) m -> p t m", p=128))

            # issue order unblocks the first projection chunk ASAP: each
            # DMA instruction costs ~650ns of SP issue time.
            dma_w(wq_sb, wq)
            dma_qt(0, split=2)
            dma_w(wk_sb, wk)
            dma_qt(1)
            dma_w(wv_sb, wv)
            nc.sync.dma_start(out=perm_sb, in_=perm[:, :])
            dma_qt(2)
            dma_qt(3)
            nc.sync.dma_start(out=cc_sb, in_=cc[:, :])
            nc.sync.dma_start(out=ss_sb, in_=ss[:, :])
            nc.sync.dma_start(out=tri_sb, in_=tri[:, :])
            nc.sync.dma_start(out=wo_sb, in_=wo[:, :])
            nc.vector.memset(v_sb, 1.0)  # ones column for rowsums survives
            nc.vector.memset(ones_f32, 1.0)
            nc.vector.tensor_copy(ones_sb, ones_f32)

            # ---- projections + rope for q and k ----
            # dst[:, cs] = ps_x * cc + (perm @ ps_x) * ss
            def make_perm_tail(sb_x, t1, dst, cs):
                def f():
                    ps_xs = psum.tile([128, SC], f32, tag="pair", bufs=3,
                                      name="ps_xs")
                    nc.tensor.matmul(ps_xs, perm_sb, sb_x, start=True,
                                     stop=True)
                    t2 = tmp.tile([128, SC], f32, tag="t2", bufs=2, name="t2")
                    nc.vector.tensor_mul(t2, ps_xs, ss_sb[:, cs])
                    nc.gpsimd.tensor_add(dst[:, cs], t1, t2)
                return f

            pending = None
            for (w_sb, dst) in ((wq_sb, qh_sb), (wk_sb, kh_sb)):
                for c in range(NSC):
                    cs = slice(c * SC, (c + 1) * SC)
                    ps_x = psum.tile([128, SC], f32, tag="pair", bufs=3,
                                     name="ps_x")
                    for t in range(NIT):
                        nc.tensor.matmul(
                            ps_x, w_sb[:, t, :], qT_sb[:, c, t, :],
                            start=(t == 0), stop=(t == NIT - 1))
                    sb_x = sbxp.tile([128, SC], bf16, tag="sbx", bufs=2,
                                     name="sb_x")
                    nc.vector.tensor_copy(sb_x, ps_x)
                    t1 = tmp.tile([128, SC], f32, tag="t1", bufs=3, name="t1")
                    nc.vector.tensor_mul(t1, ps_x, cc_sb[:, cs])
                    if pending is not None:
                        pending()
                    pending = make_perm_tail(sb_x, t1, dst, cs)

            # ---- v projection ([a, d] layout, 2 heads side by side) ----
            for t in range(NAT):
                ps_v = psum.tile([128, 128], f32, tag="one", bufs=2,
                                 name="ps_v")
                for ti in range(NIT):
                    nc.tensor.matmul(
                        ps_v,
                        qT_sb[:, t // 4, ti, (t % 4) * AT:(t % 4 + 1) * AT],
                        wv_sb[:, ti, :],
                        start=(ti == 0), stop=(ti == NIT - 1))
                if pending is not None:
                    pending()
                    pending = None
                nc.vector.tensor_copy(
                    v_sb[:, :, t, 0:HEAD_DIM],
                    ps_v.rearrange("p (h d) -> p h d", h=2))

            # ---- attention: scoresT -> exp -> attnT @ V, plus Wo drains ----
            vq = deque()   # pending attn@V / norm closures (prev iteration)
            woq = deque()  # pending Wo output-projection closures

            def drain(n_v=1, n_w=1):
                for _ in range(n_v):
                    if vq:
                        vq.popleft()()
                for _ in range(n_w):
                    if woq:
                        woq.popleft()()

            def make_wo(j):
                def mk(m):
                    def f():
                        wo_ps = psum.tile([128, 2 * SC], f32, tag="pair",
                                          bufs=3, name="wo_ps")
                        ms = slice(m * 128, (m + 1) * 128)
                        nc.tensor.matmul(wo_ps[:, 0:SC], on_sb[:, ms],
                                         wo_sb[:, 0:SC], start=True, stop=True)
                        nc.tensor.matmul(wo_ps[:, SC:2 * SC], on_sb[:, ms],
                                         wo_sb[:, SC:2 * SC], start=True,
                                         stop=True)
                        o_sb = tmp.tile([128, 2 * SC], bf16, tag="osb",
                                        bufs=3, name="o_sb")
                        if m % 2 == 0:
                            nc.scalar.copy(out=o_sb, in_=wo_ps)
                        else:
                            nc.vector.tensor_copy(o_sb, wo_ps)
                        nc.sync.dma_start(out=out[ms, :], in_=o_sb)
                    return f
                return [mk(m) for m in range(4 * j, 4 * j + 4)]

            def make_v_chunks(j, h, hp, js, at_pairs, at01, at2):
                n_full = 4 * j
                chunks = []
                state = {}

                def alloc():
                    state["ps"] = psum.tile([128, SC], f32, tag="one", bufs=2,
                                            name="ps_av")

                for p in range(n_full // 2):
                    def fchunk(p=p):
                        if p == 0:
                            alloc()
                        ps = state["ps"]
                        for k in range(2):
                            t = 2 * p + k
                            nc.tensor.matmul(
                                ps[0:HEAD_DIM + 1, :], v_sb[:, h, t, :],
                                at_pairs[p][:, k * SC:(k + 1) * SC],
                                start=(t == 0), stop=False)
                    chunks.append(fchunk)

                def dchunk():
                    if n_full == 0:
                        alloc()
                    ps = state["ps"]
                    t0 = n_full
                    nc.tensor.matmul(ps[0:HEAD_DIM + 1, 0:512],
                                     v_sb[:, h, t0 + 0, :], at01[:, 0:512],
                                     start=(n_full == 0), stop=False)
                    nc.tensor.matmul(ps[0:HEAD_DIM + 1, 128:512],
                                     v_sb[:, h, t0 + 1, :], at01[:, 512:896],
                                     start=False, stop=False)
                    nc.tensor.matmul(ps[0:HEAD_DIM + 1, 256:512],
                                     v_sb[:, h, t0 + 2, :], at2[:, 0:256],
                                     start=False, stop=False)
                    nc.tensor.matmul(ps[0:HEAD_DIM + 1, 384:512],
                                     v_sb[:, h, t0 + 3, :], at01[:, 896:1024],
                                     start=False, stop=True)
                chunks.append(dchunk)

                def norm_a():
                    ps = state["ps"]
                    avT = avp.tile([HEAD_DIM + 1, SC], f32, tag="avT", bufs=2,
                                   name="avT")
                    rec_f = recp.tile([1, SC], f32, tag="recf", bufs=2,
                                      name="rec_f")
                    with nc.allow_low_precision(reason="fp32 recip"):
                        nc.vector.reciprocal(rec_f,
                                             ps[HEAD_DIM:HEAD_DIM + 1, :])
                    nc.vector.tensor_copy(avT, ps[0:HEAD_DIM + 1, :])
                    rec = recp.tile([1, SC], f32r, tag="rec", bufs=2,
                                    name="rec")
                    nc.vector.tensor_copy(rec, rec_f)
                    state["avT"], state["rec"] = avT, rec

                def norm_b():
                    avT, rec = state["avT"], state["rec"]
                    bc = psum.tile([HEAD_DIM, SC], f32, tag="pair", bufs=3,
                                   name="bc")
                    nc.tensor.matmul(bc, ones_sb, rec, start=True,
                                     stop=True)
                    tn = tmp.tile([64, SC], bf16, tag="tn", bufs=2, name="tn")
                    nc.vector.tensor_mul(tn, avT[0:HEAD_DIM, :], bc)
                    nc.sync.dma_start(out=on_sb[hp, js], in_=tn)
                    if h == 1:
                        woq.extend(make_wo(j))
                chunks.append(norm_a)
                chunks.append(norm_b)
                return chunks

            for I in range(8):
                j, h = 3 - I // 2, I % 2
                hp = slice(64 * h, 64 * h + 64)
                base = j * SC
                js = slice(base, base + SC)
                n_full = 4 * j

                at_pairs = []
                for p in range(n_full // 2):
                    ps = psum.tile([128, 2 * SC], f32, tag="pair", bufs=3,
                                   name="ps_s")
                    for k in range(2):
                        t = 2 * p + k
                        nc.tensor.matmul(
                            ps[:, k * SC:(k + 1) * SC],
                            (kh_sb[hp, t * AT:(t + 1) * AT]),
                            (qh_sb[hp, js]), start=True, stop=True)
                    pat = atp.tile([128, 2 * SC], bf16, tag="pat", bufs=12,
                                   name="pat")
                    nc.scalar.activation(out=pat, in_=ps, func=EXP,
                                         scale=SCALE)
                    at_pairs.append(pat)
                    drain()

                # diagonal-crossing tiles: c0 full, c1/c3 share bank 1 of a
                # 2-bank tile, c2 in a 1-bank tile; narrowed to the causally
                # needed query range, triangle zeroed post-exp.
                ks = [slice((n_full + c) * AT, (n_full + c + 1) * AT)
                      for c in range(4)]
                ps01 = psum.tile([128, 2 * SC], f32, tag="pair", bufs=3,
                                 name="ps01")
                nc.tensor.matmul(ps01[:, 0:512], (kh_sb[hp, ks[0]]),
                                 (qh_sb[hp, base:base + 512]),
                                 start=True, stop=True)
                nc.tensor.matmul(ps01[:, 512:896], (kh_sb[hp, ks[1]]),
                                 (qh_sb[hp, base + 128:base + 512]),
                                 start=True, stop=True)
                nc.tensor.matmul(ps01[:, 896:1024], (kh_sb[hp, ks[3]]),
                                 (qh_sb[hp, base + 384:base + 512]),
                                 start=True, stop=True)
                at01 = atp.tile([128, 2 * SC], bf16, tag="at01", bufs=2,
                                name="at01")
                nc.scalar.activation(out=at01, in_=ps01, func=EXP, scale=SCALE)
                nc.gpsimd.tensor_mul(at01[:, 0:128], at01[:, 0:128], tri_sb)
                nc.gpsimd.tensor_mul(at01[:, 512:640], at01[:, 512:640],
                                     tri_sb)
                nc.gpsimd.tensor_mul(at01[:, 896:1024], at01[:, 896:1024],
                                     tri_sb)
                drain()

                ps2 = psum.tile([128, SC], f32, tag="one", bufs=2, name="ps2")
                nc.tensor.matmul(ps2[:, 0:256], (kh_sb[hp, ks[2]]),
                                 (qh_sb[hp, base + 256:base + 512]),
                                 start=True, stop=True)
                at2 = atp.tile([128, 256], bf16, tag="at2", bufs=2,
                               name="at2")
                nc.scalar.activation(out=at2, in_=ps2[:, 0:256], func=EXP,
                                     scale=SCALE)
                nc.gpsimd.tensor_mul(at2[:, 0:128], at2[:, 0:128], tri_sb)
                drain(n_v=2, n_w=1)

                vq.extend(make_v_chunks(j, h, hp, js, at_pairs, at01, at2))

            while vq:
                vq.popleft()()
            while woq:
                woq.popleft()()

    return nc


def _host_prep(q, Wq, Wk, Wv, Wo):
    """Build the 8 per-core input maps (numpy; bf16 via ml_dtypes)."""
    import ml_dtypes
    bf16 = ml_dtypes.bfloat16

    deint = np.concatenate([np.arange(0, HEAD_DIM, 2),
                            np.arange(1, HEAD_DIM, 2)])  # de-interleave perm
    swap = np.concatenate([np.arange(HALF, HEAD_DIM), np.arange(HALF)])

    wq_eff = Wq.reshape(IN_DIM, KV_HEADS, GROUPS, HEAD_DIM).sum(axis=2)
    wq_d = wq_eff[:, :, deint]          # [1024, 4, 64] de-interleaved
    wk_d = Wk[:, :, deint]

    # rope tables, de-interleaved layout, one head-block repeated twice
    pos = np.arange(1, S + 1, dtype=np.float64)
    thetas = 10.0 ** (-np.arange(HALF, dtype=np.float64))
    ang = pos[None, :] * thetas[:, None]          # [32, S]
    cosv, sinv = np.cos(ang), np.sin(ang)
    cc64 = np.concatenate([cosv, cosv], axis=0)   # [64, S]
    ss64 = np.concatenate([-sinv, sinv], axis=0)
    cc = np.concatenate([cc64, cc64], axis=0).astype(np.float32)   # [128, S]
    ss = np.concatenate([ss64, ss64], axis=0).astype(np.float32)

    # permutation matrix: (perm^T @ x)[r] = x[swap128[r]]
    swap128 = np.concatenate([swap, swap + 64])
    perm = np.zeros((128, 128), dtype=np.float32)
    perm[swap128, np.arange(128)] = 1.0
    perm = perm.astype(bf16)
    # keep key p of a diagonal 128x128 block for queries q >= p
    tri = np.triu(np.ones((128, 128), dtype=np.float32)).astype(bf16)

    in_maps = []
    for core in range(N_CORES):
        b, hg = core // 2, core % 2
        heads = [2 * hg, 2 * hg + 1]
        wq_c = np.concatenate([wq_d[:, h, :] for h in heads], axis=1)
        wk_c = np.concatenate([wk_d[:, h, :] for h in heads], axis=1)
        wv_c = np.concatenate([Wv[:, h, :] for h in heads], axis=1)
        in_maps.append({
            # pre-tiled [c, p, t, s]: one contiguous 1MB DMA per s-chunk
            "qT": np.ascontiguousarray(
                q[b].T.reshape(8, 128, 4, 512).transpose(2, 1, 0, 3)
                .reshape(512, 4096)).astype(bf16),
            "wq": np.ascontiguousarray(wq_c).astype(bf16),
            "wk": np.ascontiguousarray(wk_c).astype(bf16),
            "wv": np.ascontiguousarray(wv_c).astype(bf16),
            "wo": np.ascontiguousarray(Wo[hg * 128:(hg + 1) * 128, :]).astype(bf16),
            "perm": perm, "tri": tri, "cc": cc, "ss": ss,
        })
    return in_maps


def _install_ntff_hook():
    """Recreate the missing antenv.axon_hooks shim so trace=True works."""
    import sys, types
    if "antenv.axon_hooks" in sys.modules:
        return
    mod = types.ModuleType("antenv.axon_hooks")
    _hook = [None]
    mod.set_axon_ntff_profile_hook = lambda h: _hook.__setitem__(0, h)
    mod.get_axon_ntff_profile_hook = lambda: _hook[0]
    sys.modules["antenv.axon_hooks"] = mod
    try:
        if "/root/.axon_site" not in sys.path:
            sys.path.insert(0, "/root/.axon_site")
        from trn_agent_boot.trn_boot import _ntff_profile_via_ctypes
        mod.set_axon_ntff_profile_hook(
            _ntff_profile_via_ctypes("/opt/axon/libaxon_pjrt.so"))
    except Exception:
        pass


def kernel(q, mask, Wq, Wk, Wv, Wo, _dtypes=None, _trace=False):
    import sys
    if "/opt/trn_rl_repo" not in sys.path:
        sys.path.insert(0, "/opt/trn_rl_repo")
    if _trace:
        _install_ntff_hook()
    from concourse.bass_utils import run_bass_kernel_spmd

    if "nc" not in _cached:
        _cached["nc"] = _build_nc()
    nc = _cached["nc"]

    q = np.asarray(q, np.float32)
    in_maps = _host_prep(q, np.asarray(Wq, np.float32),
                         np.asarray(Wk, np.float32), np.asarray(Wv, np.float32),
                         np.asarray(Wo, np.float32))
    res = run_bass_kernel_spmd(nc, in_maps, core_ids=list(range(N_CORES)),
                               trace=_trace)
    parts = [np.asarray(r["out"], np.float32) for r in res.results]
    out = np.stack([parts[2 * b] + parts[2 * b + 1] for b in range(B)])
    if _trace:
        kernel.last_exec_time_ns = res.exec_time_ns
        kernel.last_results = res
    return out.astype(np.float32)


# revision 32
# speedup vs baseline: 1.1073x; 1.1073x over previous
"""GQA kernel for trn2, 8 NeuronCores.

Problem: nn_GroupedQueryAttention (b=4, s=2048, 16 q-heads / 4 kv-heads, d=64).
The reference's score einsum 'bghsd,bhad->bhsa' SUMS over the group axis g, and
RoPE is linear in x, so the module collapses to 4-head MHA with Wq pre-summed
over groups.

Sharding: 8 cores = (batch b in 0..3) x (head-group hg in 0..1, 2 heads each).
Each core computes its two heads' attention output and a partial product with
its 128-row slice of Wo; the host sums the two partials per batch.

Perf design (vs the f32 baseline at ~381us):
 - All matmuls at 1 cycle/row: projections, attn@V and Wo run in bf16;
   scores run f32->f32r bitcast (wide moving dim). f32 costs 4 cycles/row.
 - RoPE swap: instead of a second full projection with column-swapped weights,
   the projected chunk is copied to SBUF (bf16, Pool engine) and multiplied by
   a 128x128 permutation matrix (1 matmul of 512 rows vs 8).
 - Causal mask: diagonal-crossing tiles compute only the causally-needed
   query range (narrowed N); the per-tile 128x128 triangle is zeroed post-exp
   on DVE with a bf16 upper-tri mask. No mask-inject matmuls.
 - exp batching: score tiles for two key-tiles land side by side in a 2-bank
   PSUM tile so one ACT instruction handles 1024 columns (ACT per-instruction
   bubble is ~370ns).
 - Wo partials go PSUM -> DRAM directly (no staging copies); Wo matmuls are
   interleaved into the attention instruction stream, and attn@V runs one
   iteration behind scores (software pipelining) so PE never waits for exp.
 - softmax denominators come from a ones-column appended to V; normalization
   reciprocal is broadcast across partitions via a DRAM round-trip.
"""

from collections import deque

import numpy as np

B, S, IN_DIM = 4, 2048, 1024
Q_HEADS, KV_HEADS, HEAD_DIM = 16, 4, 64
GROUPS = Q_HEADS // KV_HEADS
HALF = HEAD_DIM // 2  # 32
N_CORES = 8
SC = 512  # s-chunk width (psum bank)
AT = 128  # a-tile width

_cached = {}


def _install_wait_splitter():
    """This walrus build accepts only ONE semaphore wait per instruction.
    Tile emits several; hoist all-but-one into standalone EventSemaphores."""
    import concourse.mybir as mybir
    import concourse.tile as tile
    from concourse._compat import not_none as nn

    if getattr(tile.TileContext, "_wait_split_installed", False):
        return

    orig_add = tile.TileContext._add_instruction

    def patched_add(self, inst):
        si = getattr(inst, "sync_info", None)
        if si is not None and si.on_wait and len(si.on_wait) > 1:
            waits = list(si.on_wait)
            for w in waits[:-1]:
                nm = self.nc.get_next_instruction_name()
                ev = mybir.InstEventSemaphore(
                    name=nm, engine=inst.engine, ins=[], outs=[],
                    sync_info=mybir.SyncInfo(on_wait=[w], on_update=[]))
                orig_add(self, ev)
            inst.sync_info = mybir.SyncInfo(
                on_wait=[waits[-1]], on_update=list(si.on_update or []))
        orig_add(self, inst)

    def patched_drain(self, tick_clock, wait_clock):
        # reimplementation of the original: same drain -> barrier -> sem-clear
        # -> barrier sequence, but the drain's (many) waits are split into
        # standalone EventSemaphores emitted BEFORE the sem clear.
        from concourse.vector_clock import ScopedClock

        nc = self.nc
        drain_wrap = nc.sync.drain()
        drain_inst = drain_wrap.ins  # BassInstruction wrapper -> mybir inst
        wait_clock.add_sem_waits(
            drain_inst, ScopedClock({None: tick_clock.global_clock}))
        bb = nn(nc.cur_bb).bb
        si = getattr(drain_inst, "sync_info", None)
        if si is not None and si.on_wait and len(si.on_wait) > 1:
            waits = list(si.on_wait)
            drain_inst.sync_info = mybir.SyncInfo(
                on_wait=[waits[0]], on_update=list(si.on_update or []))
            for w in waits[1:]:
                nm = nc.get_next_instruction_name()
                ev = mybir.InstEventSemaphore(
                    name=nm, engine=drain_inst.engine, ins=[], outs=[],
                    sync_info=mybir.SyncInfo(on_wait=[w], on_update=[]))
                nc.register_instruction(ev, overwrite=True)
                bb.add_instruction(ev)

        nc.all_engine_barrier()
        assert self.sems is not None
        popped = nc._tile_sem_poison_stack.pop()
        assert popped is self._sem_poison
        nc.clear_and_free_semaphores(list(self.sems.allocated().values()))
        nc.all_engine_barrier()

    tile.TileContext._add_instruction = patched_add
    tile.TileContext._drain_and_barrier = patched_drain
    tile.TileContext._wait_split_installed = True


def _build_nc():
    import concourse.bass as bass
    import concourse.mybir as mybir
    import concourse.tile as tile

    _install_wait_splitter()

    f32 = mybir.dt.float32
    f32r = mybir.dt.float32r
    bf16 = mybir.dt.bfloat16
    EXP = mybir.ActivationFunctionType.Exp
    SCALE = float(1.0 / np.sqrt(HEAD_DIM))

    nc = bass.Bass()

    qT = nc.declare_dram_parameter("qT", [4 * 128, 8 * 512], bf16,
                               isOutput=False)  # [c, p, t, s] pre-tiled
    wq = nc.declare_dram_parameter("wq", [IN_DIM, 128], bf16, isOutput=False)
    wk = nc.declare_dram_parameter("wk", [IN_DIM, 128], bf16, isOutput=False)
    wv = nc.declare_dram_parameter("wv", [IN_DIM, 128], bf16, isOutput=False)
    wo = nc.declare_dram_parameter("wo", [128, IN_DIM], bf16, isOutput=False)
    perm = nc.declare_dram_parameter("perm", [128, 128], bf16, isOutput=False)
    tri = nc.declare_dram_parameter("tri", [128, 128], bf16, isOutput=False)
    cc = nc.declare_dram_parameter("cc", [128, S], f32, isOutput=False)
    ss = nc.declare_dram_parameter("ss", [128, S], f32, isOutput=False)
    out = nc.declare_dram_parameter("out", [S, IN_DIM], bf16, isOutput=True)

    NSC = S // SC       # 4 s-chunks
    NAT = S // AT       # 16 a-tiles
    NIT = IN_DIM // 128  # 8 i-tiles

    def r32(ap):
        return ap.bitcast(f32r)

    with tile.TileContext(nc) as tc:
        with (
            tc.tile_pool(name="big", bufs=1) as big,
            tc.tile_pool(name="psum", bufs=1, space="PSUM") as psum,
            tc.tile_pool(name="sbx", bufs=2) as sbxp,
            tc.tile_pool(name="tmp", bufs=3) as tmp,
            tc.tile_pool(name="atp", bufs=1) as atp,
            tc.tile_pool(name="avp", bufs=2) as avp,
            tc.tile_pool(name="recp", bufs=2) as recp,
            tc.tile_pool(name="dram", bufs=2, space="DRAM") as dram,
        ):
            # ---- resident SBUF tensors ----
            qT_sb = big.tile([128, NSC, NIT, SC], bf16)
            wq_sb = big.tile([128, NIT, 128], bf16)
            wk_sb = big.tile([128, NIT, 128], bf16)
            wv_sb = big.tile([128, NIT, 128], bf16)
            wo_sb = big.tile([128, IN_DIM], bf16)
            perm_sb = big.tile([128, 128], bf16)
            tri_sb = big.tile([128, 128], bf16)
            cc_sb = big.tile([128, S], f32)
            ss_sb = big.tile([128, S], f32)
            qh_sb = big.tile([128, S], bf16)  # roped q, [2 heads x (32r|32i)], s
            kh_sb = big.tile([128, S], bf16)
            v_sb = big.tile([128, 2, NAT, HEAD_DIM + 1], bf16)  # [a, h, t, d+1]
            on_sb = big.tile([128, S], bf16)  # normalized outT, 2 heads stacked
            ones_sb = big.tile([1, HEAD_DIM], f32r)
            ones_f32 = big.tile([1, HEAD_DIM], f32)

            def dma_qt(c, split=1):
                w = NIT // split
                for u in range(split):
                    nc.sync.dma_start(
                        out=qT_sb[:, c, u * w:(u + 1) * w, :],
                        in_=qT[c * 128:(c + 1) * 128,
                               u * w * SC:(u + 1) * w * SC].rearrange(
                            "p (t s) -> p t s", s=SC))

            def dma_w(w_sb, w):
                nc.sync.dma_start(
                    out=w_sb, in_=w.rearrange("(t p) m -> p t m", p=128))

            # issue order unblocks the first projection chunk ASAP: each
            # DMA instruction costs ~650ns of SP issue time.
            dma_w(wq_sb, wq)
            dma_qt(0, split=2)
            dma_w(wk_sb, wk)
            dma_qt(1)
            dma_w(wv_sb, wv)
            nc.sync.dma_start(out=perm_sb, in_=perm[:, :])
            dma_qt(2)
            dma_qt(3)
            nc.sync.dma_start(out=cc_sb, in_=cc[:, :])
            nc.sync.dma_start(out=ss_sb, in_=ss[:, :])
            nc.sync.dma_start(out=tri_sb, in_=tri[:, :])
            nc.sync.dma_start(out=wo_sb, in_=wo[:, :])
            nc.vector.memset(v_sb, 1.0)  # ones column for rowsums survives
            nc.vector.memset(ones_f32, 1.0)
            nc.vector.tensor_copy(ones_sb, ones_f32)

            # ---- projections + rope for q and k ----
            # dst[:, cs] = ps_x * cc + (perm @ ps_x) * ss
            def make_perm_tail(sb_x, t1, dst, cs):
                def f():
                    ps_xs = psum.tile([128, SC], f32, tag="pair", bufs=3,
                                      name="ps_xs")
                    nc.tensor.matmul(ps_xs, perm_sb, sb_x, start=True,
                                     stop=True)
                    t2 = tmp.tile([128, SC], f32, tag="t2", bufs=2, name="t2")
                    nc.vector.tensor_mul(t2, ps_xs, ss_sb[:, cs])
                    nc.gpsimd.tensor_add(dst[:, cs], t1, t2)
                return f

            pending = None
            for (w_sb, dst) in ((wq_sb, qh_sb), (wk_sb, kh_sb)):
                for c in range(NSC):
                    cs = slice(c * SC, (c + 1) * SC)
                    ps_x = psum.tile([128, SC], f32, tag="pair", bufs=3,
                                     name="ps_x")
                    for t in range(NIT):
                        nc.tensor.matmul(
                            ps_x, w_sb[:, t, :], qT_sb[:, c, t, :],
                            start=(t == 0), stop=(t == NIT - 1))
                    sb_x = sbxp.tile([128, SC], bf16, tag="sbx", bufs=2,
                                     name="sb_x")
                    nc.vector.tensor_copy(sb_x, ps_x)
                    t1 = tmp.tile([128, SC], f32, tag="t1", bufs=3, name="t1")
                    nc.vector.tensor_mul(t1, ps_x, cc_sb[:, cs])
                    if pending is not None:
                        pending()
                    pending = make_perm_tail(sb_x, t1, dst, cs)

            # ---- v projection ([a, d] layout, 2 heads side by side) ----
            for t in range(NAT):
                ps_v = psum.tile([128, 128], f32, tag="one", bufs=2,
                                 name="ps_v")
                for ti in range(NIT):
                    nc.tensor.matmul(
                        ps_v,
                        qT_sb[:, t // 4, ti, (t % 4) * AT:(t % 4 + 1) * AT],
                        wv_sb[:, ti, :],
                        start=(ti == 0), stop=(ti == NIT - 1))
                if pending is not None:
                    pending()
                    pending = None
                nc.vector.tensor_copy(
                    v_sb[:, :, t, 0:HEAD_DIM],
                    ps_v.rearrange("p (h d) -> p h d", h=2))

            # ---- attention: scoresT -> exp -> attnT @ V, plus Wo drains ----
            vq = deque()   # pending attn@V / norm closures (prev iteration)
            woq = deque()  # pending Wo output-projection closures

            def drain(n_v=1, n_w=1):
                for _ in range(n_v):
                    if vq:
                        vq.popleft()()
                for _ in range(n_w):
                    if woq:
                        woq.popleft()()

            def make_wo(j):
                def mk(m):
                    def f():
                        wo_ps = psum.tile([128, 2 * SC], f32, tag="pair",
                                          bufs=3, name="wo_ps")
                        ms = slice(m * 128, (m + 1) * 128)
                        nc.tensor.matmul(wo_ps[:, 0:SC], on_sb[:, ms],
                                         wo_sb[:, 0:SC], start=True, stop=True)
                        nc.tensor.matmul(wo_ps[:, SC:2 * SC], on_sb[:, ms],
                                         wo_sb[:, SC:2 * SC], start=True,
                                         stop=True)
                        o_sb = tmp.tile([128, 2 * SC], bf16, tag="osb",
                                        bufs=3, name="o_sb")
                        if m % 2 == 0:
                            nc.scalar.copy(out=o_sb, in_=wo_ps)
                        else:
                            nc.vector.tensor_copy(o_sb, wo_ps)
                        nc.sync.dma_start(out=out[ms, :], in_=o_sb)
                    return f
                return [mk(m) for m in range(4 * j, 4 * j + 4)]

            def make_v_chunks(j, h, hp, js, at_pairs, at01, at2):
                n_full = 4 * j
                chunks = []
                state = {}

                def alloc():
                    state["ps"] = psum.tile([128, SC], f32, tag="one", bufs=2,
                                            name="ps_av")

                for p in range(n_full // 2):
                    def fchunk(p=p):
                        if p == 0:
                            alloc()
                        ps = state["ps"]
                        for k in range(2):
                            t = 2 * p + k
                            nc.tensor.matmul(
                                ps[0:HEAD_DIM + 1, :], v_sb[:, h, t, :],
                                at_pairs[p][:, k * SC:(k + 1) * SC],
                                start=(t == 0), stop=False)
                    chunks.append(fchunk)

                def dchunk():
                    if n_full == 0:
                        alloc()
                    ps = state["ps"]
                    t0 = n_full
                    nc.tensor.matmul(ps[0:HEAD_DIM + 1, 0:512],
                                     v_sb[:, h, t0 + 0, :], at01[:, 0:512],
                                     start=(n_full == 0), stop=False)
                    nc.tensor.matmul(ps[0:HEAD_DIM + 1, 128:512],
                                     v_sb[:, h, t0 + 1, :], at01[:, 512:896],
                                     start=False, stop=False)
                    nc.tensor.matmul(ps[0:HEAD_DIM + 1, 256:512],
                                     v_sb[:, h, t0 + 2, :], at2[:, 0:256],
                                     start=False, stop=False)
                    nc.tensor.matmul(ps[0:HEAD_DIM + 1, 384:512],
                                     v_sb[:, h, t0 + 3, :], at01[:, 896:1024],
                                     start=False, stop=True)
                chunks.append(dchunk)

                def norm_a():
                    ps = state["ps"]
                    avT = avp.tile([HEAD_DIM + 1, SC], f32, tag="avT", bufs=2,
                                   name="avT")
                    nc.vector.tensor_copy(avT, ps[0:HEAD_DIM + 1, :])
                    rec_f = recp.tile([1, SC], f32, tag="recf", bufs=2,
                                      name="rec_f")
                    with nc.allow_low_precision(reason="fp32 recip"):
                        nc.vector.reciprocal(rec_f,
                                             avT[HEAD_DIM:HEAD_DIM + 1, :])
                    rec = recp.tile([1, SC], f32r, tag="rec", bufs=2,
                                    name="rec")
                    nc.vector.tensor_copy(rec, rec_f)
                    state["avT"], state["rec"] = avT, rec

                def norm_b():
                    avT, rec = state["avT"], state["rec"]
                    bc = psum.tile([HEAD_DIM, SC], f32, tag="pair", bufs=3,
                                   name="bc")
                    nc.tensor.matmul(bc, ones_sb, rec, start=True,
                                     stop=True)
                    tn = tmp.tile([64, SC], bf16, tag="tn", bufs=2, name="tn")
                    nc.vector.tensor_mul(tn, avT[0:HEAD_DIM, :], bc)
                    nc.sync.dma_start(out=on_sb[hp, js], in_=tn)
                    if h == 1:
                        woq.extend(make_wo(j))
                chunks.append(norm_a)
                chunks.append(norm_b)
                return chunks

            for I in range(8):
                j, h = 3 - I // 2, I % 2
                hp = slice(64 * h, 64 * h + 64)
                base = j * SC
                js = slice(base, base + SC)
                n_full = 4 * j

                at_pairs = []
                for p in range(n_full // 2):
                    ps = psum.tile([128, 2 * SC], f32, tag="pair", bufs=3,
                                   name="ps_s")
                    for k in range(2):
                        t = 2 * p + k
                        nc.tensor.matmul(
                            ps[:, k * SC:(k + 1) * SC],
                            (kh_sb[hp, t * AT:(t + 1) * AT]),
                            (qh_sb[hp, js]), start=True, stop=True)
                    pat = atp.tile([128, 2 * SC], bf16, tag="pat", bufs=12,
                                   name="pat")
                    nc.scalar.activation(out=pat, in_=ps, func=EXP,
                                         scale=SCALE)
                    at_pairs.append(pat)
                    drain()

                # diagonal-crossing tiles: c0 full, c1/c3 share bank 1 of a
                # 2-bank tile, c2 in a 1-bank tile; narrowed to the causally
                # needed query range, triangle zeroed post-exp.
                ks = [slice((n_full + c) * AT, (n_full + c + 1) * AT)
                      for c in range(4)]
                ps01 = psum.tile([128, 2 * SC], f32, tag="pair", bufs=3,
                                 name="ps01")
                nc.tensor.matmul(ps01[:, 0:512], (kh_sb[hp, ks[0]]),
                                 (qh_sb[hp, base:base + 512]),
                                 start=True, stop=True)
                nc.tensor.matmul(ps01[:, 512:896], (kh_sb[hp, ks[1]]),
                                 (qh_sb[hp, base + 128:base + 512]),
                                 start=True, stop=True)
                nc.tensor.matmul(ps01[:, 896:1024], (kh_sb[hp, ks[3]]),
                                 (qh_sb[hp, base + 384:base + 512]),
                                 start=True, stop=True)
                at01 = atp.tile([128, 2 * SC], bf16, tag="at01", bufs=2,
                                name="at01")
                nc.scalar.activation(out=at01, in_=ps01, func=EXP, scale=SCALE)
                nc.gpsimd.tensor_mul(at01[:, 0:128], at01[:, 0:128], tri_sb)
                nc.gpsimd.tensor_mul(at01[:, 512:640], at01[:, 512:640],
                                     tri_sb)
                nc.gpsimd.tensor_mul(at01[:, 896:1024], at01[:, 896:1024],
                                     tri_sb)
                drain()

                ps2 = psum.tile([128, SC], f32, tag="one", bufs=2, name="ps2")
                nc.tensor.matmul(ps2[:, 0:256], (kh_sb[hp, ks[2]]),
                                 (qh_sb[hp, base + 256:base + 512]),
                                 start=True, stop=True)
                at2 = atp.tile([128, 256], bf16, tag="at2", bufs=2,
                               name="at2")
                nc.scalar.activation(out=at2, in_=ps2[:, 0:256], func=EXP,
                                     scale=SCALE)
                nc.gpsimd.tensor_mul(at2[:, 0:128], at2[:, 0:128], tri_sb)
                drain(n_v=2, n_w=1)

                vq.extend(make_v_chunks(j, h, hp, js, at_pairs, at01, at2))

            while vq:
                vq.popleft()()
            while woq:
                woq.popleft()()

    return nc


def _host_prep(q, Wq, Wk, Wv, Wo):
    """Build the 8 per-core input maps (numpy; bf16 via ml_dtypes)."""
    import ml_dtypes
    bf16 = ml_dtypes.bfloat16

    deint = np.concatenate([np.arange(0, HEAD_DIM, 2),
                            np.arange(1, HEAD_DIM, 2)])  # de-interleave perm
    swap = np.concatenate([np.arange(HALF, HEAD_DIM), np.arange(HALF)])

    wq_eff = Wq.reshape(IN_DIM, KV_HEADS, GROUPS, HEAD_DIM).sum(axis=2)
    wq_d = wq_eff[:, :, deint]          # [1024, 4, 64] de-interleaved
    wk_d = Wk[:, :, deint]

    # rope tables, de-interleaved layout, one head-block repeated twice
    pos = np.arange(1, S + 1, dtype=np.float64)
    thetas = 10.0 ** (-np.arange(HALF, dtype=np.float64))
    ang = pos[None, :] * thetas[:, None]          # [32, S]
    cosv, sinv = np.cos(ang), np.sin(ang)
    cc64 = np.concatenate([cosv, cosv], axis=0)   # [64, S]
    ss64 = np.concatenate([-sinv, sinv], axis=0)
    cc = np.concatenate([cc64, cc64], axis=0).astype(np.float32)   # [128, S]
    ss = np.concatenate([ss64, ss64], axis=0).astype(np.float32)

    # permutation matrix: (perm^T @ x)[r] = x[swap128[r]]
    swap128 = np.concatenate([swap, swap + 64])
    perm = np.zeros((128, 128), dtype=np.float32)
    perm[swap128, np.arange(128)] = 1.0
    perm = perm.astype(bf16)
    # keep key p of a diagonal 128x128 block for queries q >= p
    tri = np.triu(np.ones((128, 128), dtype=np.float32)).astype(bf16)

    in_maps = []
    for core in range(N_CORES):
        b, hg = core // 2, core % 2
        heads = [2 * hg, 2 * hg + 1]
        wq_c = np.concatenate([wq_d[:, h, :] for h in heads], axis=1)
        wk_c = np.concatenate([wk_d[:, h, :] for h in heads], axis=1)
        wv_c = np.concatenate([Wv[:, h, :] for h in heads], axis=1)
        in_maps.append({
            # pre-tiled [c, p, t, s]: one contiguous 1MB DMA per s-chunk
            "qT": np.ascontiguousarray(
                q[b].T.reshape(8, 128, 4, 512).transpose(2, 1, 0, 3)
                .reshape(512, 4096)).astype(bf16),
            "wq": np.ascontiguousarray(wq_c).astype(bf16),
            "wk": np.ascontiguousarray(wk_c).astype(bf16),
            "wv": np.ascontiguousarray(wv_c).astype(bf16),
            "wo": np.ascontiguousarray(Wo[hg * 128:(hg + 1) * 128, :]).astype(bf16),
            "perm": perm, "tri": tri, "cc": cc, "ss": ss,
        })
    return in_maps


def _install_ntff_hook():
    """Recreate the missing antenv.axon_hooks shim so trace=True works."""
    import sys, types
    if "antenv.axon_hooks" in sys.modules:
        return
    mod = types.ModuleType("antenv.axon_hooks")
    _hook = [None]
    mod.set_axon_ntff_profile_hook = lambda h: _hook.__setitem__(0, h)
    mod.get_axon_ntff_profile_hook = lambda: _hook[0]
    sys.modules["antenv.axon_hooks"] = mod
    try:
        if "/root/.axon_site" not in sys.path:
            sys.path.insert(0, "/root/.axon_site")
        from trn_agent_boot.trn_boot import _ntff_profile_via_ctypes
        mod.set_axon_ntff_profile_hook(
            _ntff_profile_via_ctypes("/opt/axon/libaxon_pjrt.so"))
    except Exception:
        pass


def kernel(q, mask, Wq, Wk, Wv, Wo, _dtypes=None, _trace=False):
    import sys
    if "/opt/trn_rl_repo" not in sys.path:
        sys.path.insert(0, "/opt/trn_rl_repo")
    if _trace:
        _install_ntff_hook()
    from concourse.bass_utils import run_bass_kernel_spmd

    if "nc" not in _cached:
        _cached["nc"] = _build_nc()
    nc = _cached["nc"]

    q = np.asarray(q, np.float32)
    in_maps = _host_prep(q, np.asarray(Wq, np.float32),
                         np.asarray(Wk, np.float32), np.asarray(Wv, np.float32),
                         np.asarray(Wo, np.float32))
    res = run_bass_kernel_spmd(nc, in_maps, core_ids=list(range(N_CORES)),
                               trace=_trace)
    parts = [np.asarray(r["out"], np.float32) for r in res.results]
    out = np.stack([parts[2 * b] + parts[2 * b + 1] for b in range(B)])
    if _trace:
        kernel.last_exec_time_ns = res.exec_time_ns
        kernel.last_results = res
    return out.astype(np.float32)
